# revision 24
# baseline (speedup 1.0000x reference)
"""Trainium2 Bass kernel for the sketched-attention RS_SM op.

Reference semantics (per (b,h) pair):
    X  = concat([Q, K], axis=seq)                      # [4096, 64]
    XS = gather of 1024 landmark rows of X             # [m=4, d=256, 64]
    AS[n, d] = sum_m sign[m, d] * exp(X[n] . XS[m, d]) # [4096, 256]

Sharding: 16 (b,h) pairs over 8 cores = 2 pairs/core, no cross-core comms.

The wall is the PSUM->SBUF elementwise flux (8.4M exp reads + 2.1M AS
copies per core through ScalarE @1.2G + VectorE @0.96G elem/s/partition
~= 44us); the PE is kept well under it:
  MM1  : row-tiled (64x128) PE mode -- pair pr contracts over its own
         64 X-feature partitions at tile_position (64*pr, 0), so the
         two pairs' matmuls stream CONCURRENTLY on the two row tiles
         and each tile's LDWEIGHTS overlaps the other row group's
         matmuls.  PSUM [128 lmk, 1024] f32 tiles (2 chunks).
  exp  : one [128, 1024] op per PSUM tile, alternated ScalarE exact
         Exp / VectorE Schraudolph fast-exp (tensor_scalar x*A + B,
         int16 out whose bits ARE bf16 exp(x)); signs live in W.
  MM2  : col-tiled (128x32) signed m-reduction, lhsT = sign-delta W
         [128, 32], 4 col-tiles concurrent -> PSUM [128 d, 1024] both
         halves.  MM1/MM2 run in 2-token-chunk phases so the PE pays
         only 8 tiling-mode-switch drains per body instead of 32.
  copy : one [128, 1024] PSUM->SBUF bf16 op per (pair, t), in the same
         balanced ScalarE/VectorE unit-op rotation as the exps.
  out  : two contiguous DMAs per pair: [128, 2, 2048] bf16.

Landmark order is permuted (host-side) so chunk c holds (m, dl) for
d = 32c + dl: partition p = 32*m + dl.  W[32m+dl, 32c+dl] = sign[m, 32c+dl].
Output HBM layout is [pair, 128 p, 2 h, 4096 n] bf16 with d = 128h + p;
the host decodes to [4096, 256] f32 at unshard.

All three device inputs (X^T | landmarks^T | W) are packed into one
[128, INW] bf16 array so a single DMA (one semaphore lane) feeds the PE
-- multiple DMA waits on one fused-LDW matmul overflow its sync-wait
slots.  With KERNEL_MM1_FP8=1 the X^T/landmark sections are instead
fp8e4 bytes (2 contract k-tiles per partition) and MM1 runs in
DoubleRow mode at 0.5 PE cycles/row.

KERNEL_ITERS repeats the body (same in/out) inside one NEFF so the
benchmark can measure the marginal per-iteration hardware time,
excluding the multi-ms host->axon dispatch overhead.
"""

import math
import os
import sys
import types
from contextlib import ExitStack

import numpy as np

sys.path.insert(0, "/opt/trn_rl_repo")

# The axon client in this container lacks the NTFF profile hook module;
# provide a stub so bass_utils' trace path degrades gracefully.
try:
    import antenv.axon_hooks  # noqa: F401
except ImportError:
    _stub = types.ModuleType("antenv.axon_hooks")
    _stub.get_axon_ntff_profile_hook = lambda: None
    sys.modules["antenv.axon_hooks"] = _stub

import concourse.bacc as bacc
import concourse.bass as bass
import concourse.mybir as mybir
import concourse.tile as tile

B, H, N, P = 2, 8, 2048, 64
M, D = 4, 256
SEQ2 = 2 * N                      # 4096 tokens per pair
NCORES = 8
PAIRS = (B * H) // NCORES         # 2 pairs per core
L = M * D                         # 1024 landmarks per pair
TCH = 512                         # token chunk (matmul moving dim)
NT = SEQ2 // TCH                  # 8 token chunks
NC_ = 8                           # landmark chunks of 128
INW = SEQ2 + PAIRS * L + D        # packed input width: xt | lt0 | lt1 | w
F32 = mybir.dt.float32
BF16 = mybir.dt.bfloat16
I16 = mybir.dt.int16

# Schraudolph fast-exp in bf16-bit domain: bits16 = round(x*A + B);
# bitcast16 -> bf16 ~= exp(x).  C trades max vs rms error.
EXP_A = 128.0 / math.log(2.0)
EXP_C = float(os.environ.get("KERNEL_EXP_C", "5.0"))
EXP_B = 127.0 * 128.0 - EXP_C

# per-body elementwise unit-ops, all [128, 1024] PSUM->SBUF and
# near-equal cost: 64 exps + 16 AS copies.  SC_UNITS of them run on
# ScalarE (rate ~1.0 Gelem/s/partition), the rest on VectorE (~0.83);
# 44/80 equalizes the two engines' finish times.
N_UNITS = 80
SC_UNITS = int(os.environ.get("KERNEL_SC_UNITS", "44"))

# timing-only bisection probes: "no_mm1" / "no_mm2" / "no_exp" / "no_dma"
PROBE = os.environ.get("KERNEL_PROBE", "")

# token chunks per MM1/MM2 phase block: more = fewer PE tiling-mode
# switch drains, but more exp-tile SBUF and a longer MM2 tail.
TBLOCK = int(os.environ.get("KERNEL_TBLOCK", "2"))

# split MM1's 64-feature contraction into two K=32 row tiles per pair
# (4 concurrent PE row tiles instead of 2)
MM1_K32 = os.environ.get("KERNEL_MM1_K32", "0") == "1"

# MM1 in fp8e4 (e4m3) DoubleRow mode: 2 contract rows packed per
# partition, 0.5 PE cycles/row.  X/landmarks quantize to e4m3 (values
# ~N(0, 1/64), dot std 1/8 -> ~0.6% incoherent exp error).  The packed
# input then carries fp8 X^T/landmarks instead of bf16 ones.
MM1_FP8 = os.environ.get("KERNEL_MM1_FP8", "0") == "1"
FP8 = mybir.dt.float8e4
# fp8-mode packed layout (bf16-slot columns): xt8 | lt8 | w
INW_FP8 = SEQ2 + L + D

_nc_cache = {}


def _build_nc(iters: int = 1):
    nc = bacc.Bacc(
        "TRN2", target_bir_lowering=False, debug=False, num_devices=NCORES,
    )

    inw = INW_FP8 if MM1_FP8 else INW
    inp = nc.dram_tensor("inp", [128, inw], BF16, kind="ExternalInput")
    out = nc.dram_tensor("out", [PAIRS, 128, 2 * SEQ2], BF16,
                         kind="ExternalOutput")

    with tile.TileContext(nc) as tc, ExitStack() as ctx:
        inp_pool = ctx.enter_context(tc.tile_pool(name="inp", bufs=2))
        # MM1 psum: [128, 1024] f32 = 2 banks, bufs=3 -> 6 banks
        eps_pool = ctx.enter_context(
            tc.tile_pool(name="eps", bufs=3, space="PSUM"))
        # MM2 psum: [128, 1024] f32 = 2 banks, bufs=1 -> 2 banks (8 total)
        asps_pool = ctx.enter_context(
            tc.tile_pool(name="asps", bufs=1, space="PSUM"))
        esb_pool = ctx.enter_context(
            tc.tile_pool(name="esb", bufs=2 * (TBLOCK + 1)))
        assb_pool = ctx.enter_context(tc.tile_pool(name="assb", bufs=2))

        def body():
            inp_sb = inp_pool.tile([128, inw], BF16, tag="inp")
            nc.sync.dma_start(inp_sb[:], inp[:])
            if MM1_FP8:
                # fp8 sections hold pair pr on partitions 32pr..32pr+32,
                # 2 contract k-tiles per partition (feature = kt*32 + p)
                xt8 = [inp_sb[32 * p:32 * (p + 1), 0:SEQ2].bitcast(FP8)
                       .rearrange("p (k n) -> p k n", k=2)
                       for p in range(PAIRS)]
                lt8 = [inp_sb[32 * p:32 * (p + 1), SEQ2:SEQ2 + L]
                       .bitcast(FP8).rearrange("p (k l) -> p k l", k=2)
                       for p in range(PAIRS)]
                w_sb = inp_sb[:, SEQ2 + L:INW_FP8]
            else:
                xt_sb = inp_sb[:, 0:SEQ2]
                lt_sb = [inp_sb[:, SEQ2 + L * p:SEQ2 + L * (p + 1)]
                         for p in range(PAIRS)]
                w_sb = inp_sb[:, SEQ2 + PAIRS * L:INW]

            # unit-op scheduler: the 80 elementwise ops per body (64 exp +
            # 16 AS copies, all [128, 1024] PSUM->SBUF, near-equal cost)
            # alternate ScalarE/VectorE so both engines drain the PSUM flux
            # together.  ScalarE runs exact Exp; VectorE runs the
            # Schraudolph fast-exp (signs live in W, not in the exp).
            unit = {"k": 0}

            def on_scalar():
                k = unit["k"]
                unit["k"] += 1
                return (k * SC_UNITS) // N_UNITS != \
                    ((k + 1) * SC_UNITS) // N_UNITS

            as_tiles = {}
            e_tiles = {}

            def mm1_phase(t):
                """Row-tiled (64x128) MM1 for both pairs of token chunk t.
                Pair pr contracts over its own 64 X-feature partitions at
                tile_position (64*pr, 0); the two row tiles stream
                concurrently and their LDWEIGHTS overlap the other row
                group's matmuls."""
                if t == 0:
                    for pr in range(PAIRS):
                        as_tiles[pr] = assb_pool.tile(
                            [128, 2 * SEQ2], BF16, tag="assb", name="as_sb")
                for pr in range(PAIRS):
                    e_tiles[(pr, t)] = esb_pool.tile(
                        [128, NC_ * TCH], BF16, tag="esb", name="e_sb")
                for cg in range(4):            # chunk-group: 2 landmark chunks
                    ps = {}
                    for pr in range(PAIRS):
                        ps[pr] = eps_pool.tile([128, 2 * TCH], F32,
                                               tag="eps", name="e_ps")
                    # interleave pairs so the row tiles stay concurrent
                    for ci in range(2):
                        c = 2 * cg + ci
                        for pr in range(PAIRS):
                            if PROBE == "no_mm1" and ci > 0:
                                continue
                            dst = ps[pr][:, ci * TCH:(ci + 1) * TCH]
                            if MM1_FP8:
                                nc.tensor.matmul(
                                    dst,
                                    lhsT=lt8[pr][:, :,
                                                 128 * c:128 * (c + 1)],
                                    rhs=xt8[pr][:, :,
                                                t * TCH:(t + 1) * TCH],
                                    start=True, stop=True,
                                    perf_mode=mybir.MatmulPerfMode.DoubleRow,
                                    tile_position=(32 * pr, 0),
                                )
                            elif MM1_K32:
                                for kk in range(2):
                                    rows = slice(64 * pr + 32 * kk,
                                                 64 * pr + 32 * (kk + 1))
                                    nc.tensor.matmul(
                                        dst,
                                        lhsT=lt_sb[pr][rows,
                                                       128 * c:128 * (c + 1)],
                                        rhs=xt_sb[rows,
                                                  t * TCH:(t + 1) * TCH],
                                        start=(kk == 0), stop=(kk == 1),
                                        tile_position=(64 * pr + 32 * kk, 0),
                                    )
                            else:
                                rows = slice(64 * pr, 64 * (pr + 1))
                                nc.tensor.matmul(
                                    dst,
                                    lhsT=lt_sb[pr][rows,
                                                   128 * c:128 * (c + 1)],
                                    rhs=xt_sb[rows, t * TCH:(t + 1) * TCH],
                                    start=True, stop=True,
                                    tile_position=(64 * pr, 0),
                                )
                    for pr in range(PAIRS):
                        dst = e_tiles[(pr, t)][:,
                                               2 * cg * TCH:2 * (cg + 1) * TCH]
                        if PROBE == "no_exp":
                            unit["k"] += 1
                            continue
                        if on_scalar():
                            nc.scalar.activation(
                                dst, ps[pr][:],
                                mybir.ActivationFunctionType.Exp,
                            )
                        else:
                            nc.vector.tensor_scalar(
                                dst.bitcast(I16),
                                ps[pr][:],
                                EXP_A, EXP_B,
                                mybir.AluOpType.mult, mybir.AluOpType.add,
                            )

            def mm2_phase(t):
                """Col-tiled (128x32) signed m-reduction for both pairs of
                token chunk t, then one wide AS copy per pair."""
                for pr in range(PAIRS):
                    e_sb = e_tiles.pop((pr, t))
                    as_sb = as_tiles[pr]
                    as_hview = as_sb[:].rearrange("p (h n) -> p h n", h=2)
                    as_ps = asps_pool.tile([128, 2 * TCH], F32, tag="asps",
                                           name="as_ps")
                    for half in range(2):
                        for j in range(4):
                            c = 4 * half + j
                            if PROBE == "no_mm2" and j > 0:
                                continue
                            nc.tensor.matmul(
                                as_ps[32 * j:32 * (j + 1),
                                      half * TCH:(half + 1) * TCH],
                                lhsT=w_sb[:, 32 * c:32 * (c + 1)],
                                rhs=e_sb[:, c * TCH:(c + 1) * TCH],
                                start=True, stop=True,
                                tile_position=(0, 32 * j),
                            )
                    # both halves leave PSUM in one wide op; dst is the
                    # (h, n) strided view of the pair's output tile
                    dst = as_hview[:, :, t * TCH:(t + 1) * TCH]
                    src = as_ps[:].rearrange("p (h n) -> p h n", h=2)
                    if on_scalar():
                        nc.scalar.copy(dst, src)
                    else:
                        nc.vector.tensor_copy(dst, src)
                    if PROBE == "no_dma":
                        continue
                    if t == NT // 2 - 1:
                        nc.sync.dma_start(
                            out[pr].rearrange("p (h n) -> p h n", h=2)[
                                :, :, 0:(NT // 2) * TCH],
                            as_hview[:, :, 0:(NT // 2) * TCH])
                    elif t == NT - 1:
                        nc.sync.dma_start(
                            out[pr].rearrange("p (h n) -> p h n", h=2)[
                                :, :, (NT // 2) * TCH:SEQ2],
                            as_hview[:, :, (NT // 2) * TCH:SEQ2])

            # TBLOCK-token-chunk blocks: MM1 (row-tiled) then MM2
            # (col-tiled) -> 2*NT/TBLOCK PE tiling-mode switches per body
            for tb in range(NT // TBLOCK):
                for i in range(TBLOCK):
                    mm1_phase(tb * TBLOCK + i)
                for i in range(TBLOCK):
                    mm2_phase(tb * TBLOCK + i)

        if iters == 1:
            body()
        else:
            # unroll 4 bodies per hardware-loop iteration so the per-
            # iteration all-engine barrier amortizes over 4 kernel runs
            assert iters % 4 == 0, iters
            with tc.For_i(0, iters // 4):
                for _u in range(4):
                    body()
    nc.compile()
    return nc


def _get_nc(iters: int = 1):
    key = (iters, SC_UNITS, TBLOCK, MM1_K32)
    if key not in _nc_cache:
        _nc_cache[key] = _build_nc(iters)
    return _nc_cache[key]


_runner_cache = {}


def _get_runner(iters: int = 1):
    """Build (once) a jitted shard_map callable over the 8 cores, mirroring
    bass2jax.run_bass_via_pjrt but cached so repeat calls don't re-trace."""
    key = (iters, SC_UNITS, TBLOCK, MM1_K32)
    if key in _runner_cache:
        return _runner_cache[key]
    import jax
    from jax.sharding import Mesh, PartitionSpec
    try:
        from jax.experimental.shard_map import shard_map
    except ImportError:
        from jax.shard_map import shard_map  # newer jax
    from concourse import bass2jax as b2j

    b2j.install_neuronx_cc_hook()
    nc = _get_nc(iters)

    partition_name = (
        nc.partition_id_tensor.name if nc.partition_id_tensor else None
    )
    in_names, out_names, out_avals, zero_shapes = [], [], [], []
    for alloc in nc.m.functions[0].allocations:
        if not isinstance(alloc, mybir.MemoryLocationSet):
            continue
        name = alloc.memorylocations[0].name
        if alloc.kind == "ExternalInput":
            if name != partition_name:
                in_names.append(name)
        elif alloc.kind == "ExternalOutput":
            out_names.append(name)
            shape = tuple(alloc.tensor_shape)
            dtype = mybir.dt.np(alloc.dtype)
            out_avals.append(jax.core.ShapedArray(shape, dtype))
            zero_shapes.append((shape, dtype))
    n_params = len(in_names)
    n_outs = len(out_avals)
    all_names = list(in_names) + list(out_names)
    if partition_name is not None:
        all_names.append(partition_name)
    donate = tuple(range(n_params, n_params + n_outs))

    def _body(*args):
        operands = list(args)
        if partition_name is not None:
            operands.append(b2j.partition_id_tensor())
        outs = b2j._bass_exec_p.bind(
            *operands,
            out_avals=tuple(out_avals),
            in_names=tuple(all_names),
            out_names=tuple(out_names),
            lowering_input_output_aliases=(),
            sim_require_finite=True,
            sim_require_nnan=True,
            nc=nc,
        )
        return tuple(outs)

    devices = jax.devices()[:NCORES]
    mesh = Mesh(np.asarray(devices), ("core",))
    in_specs = (PartitionSpec("core"),) * (n_params + n_outs)
    out_specs = (PartitionSpec("core"),) * n_outs
    sharded = jax.jit(
        shard_map(_body, mesh=mesh, in_specs=in_specs,
                  out_specs=out_specs, check_rep=False),
        donate_argnums=donate,
        keep_unused=True,
    )
    runner = {
        "jit": sharded, "in_names": in_names, "out_names": out_names,
        "out_avals": out_avals, "zero_shapes": zero_shapes, "mesh": mesh,
    }
    _runner_cache[key] = runner
    return runner


def _run_cores(in_maps):
    runner = _get_runner(1)
    concat_in = [
        np.concatenate([in_maps[c][name] for c in range(NCORES)], axis=0)
        for name in runner["in_names"]
    ]
    concat_zeros = [
        np.zeros((NCORES * s[0], *s[1:]), d) for (s, d) in runner["zero_shapes"]
    ]
    out_arrs = runner["jit"](*concat_in, *concat_zeros)
    results = []
    for c in range(NCORES):
        results.append({
            name: np.asarray(out_arrs[i]).reshape(
                NCORES, *runner["out_avals"][i].shape)[c]
            for i, name in enumerate(runner["out_names"])
        })
    return results


def _make_timer(runner, in_maps, calls=12):
    """Stage inputs once; return (one_timed_pass, first_output).  A pass
    enqueues `calls` executions back-to-back and blocks once; the
    per-call slope removes the blocking round-trip latency."""
    import time as _time
    import jax
    from jax.sharding import NamedSharding, PartitionSpec
    mesh = runner["mesh"]
    shard = NamedSharding(mesh, PartitionSpec("core"))
    concat_in = [
        np.concatenate([in_maps[c][name] for c in range(NCORES)], axis=0)
        for name in runner["in_names"]
    ]
    dev_in = [jax.device_put(a, shard) for a in concat_in]
    fn = runner["jit"]

    def zeros_dev():
        return [
            jax.device_put(np.zeros((NCORES * s[0], *s[1:]), d), shard)
            for (s, d) in runner["zero_shapes"]
        ]

    out = fn(*dev_in, *zeros_dev())
    jax.block_until_ready(out)
    first_out = [np.asarray(o) for o in out]

    def one_pass():
        zsets = [zeros_dev() for _ in range(calls)]
        jax.block_until_ready(zsets)
        outs = []
        t0 = _time.perf_counter()
        for z in zsets:
            outs.append(fn(*dev_in, *z))
        jax.block_until_ready(outs)
        t1 = _time.perf_counter()
        return (t1 - t0) / calls

    return one_pass, first_out


def benchmark(in_maps, iters_lo=64, iters_hi=256, calls=8, reps=25):
    """Hardware exec time per kernel instance: the same kernel body is
    repeated N times via a hardware For_i loop inside one NEFF; the
    marginal cost (T(iters_hi) - T(iters_lo)) / (iters_hi - iters_lo)
    is pure device execution.  Both ends are loop NEFFs with identical
    dispatch structure, so the host/axon dispatch overhead cancels; the
    lo/hi timing passes are interleaved and the median of the paired
    slopes taken, so slow drift in ambient terminal load (the dominant
    noise source) cancels too."""
    rl = _get_runner(iters_lo)
    rh = _get_runner(iters_hi)
    pass_lo, outl = _make_timer(rl, in_maps, calls=calls)
    pass_hi, outh = _make_timer(rh, in_maps, calls=calls)
    if not PROBE:
        for a, b in zip(outl, outh):
            assert np.array_equal(a, b), "looped kernel output mismatch"
    slopes, tls, ths = [], [], []
    for _ in range(reps):
        tl = pass_lo()
        th = pass_hi()
        tls.append(tl)
        ths.append(th)
        slopes.append((th - tl) / (iters_hi - iters_lo))
    # Individual paired slopes can be corrupted by multi-ms drift in the
    # axon dispatch overhead between a pair's lo and hi passes (observed
    # from -50us to +86us on identical kernels), and the device is
    # time-shared so most reps read the contended rate.  Take the min
    # slope that is above half the median: drift artifacts (far-low or
    # negative) are excluded, while a genuinely quiet rep -- which sits
    # within 2x of the contended cluster -- is kept.
    med = float(np.median(slopes))
    ok = [s for s in slopes if s > 0.5 * med] or slopes
    hw = float(np.min(ok))
    print(f"slopes us: {[round(s * 1e6, 1) for s in sorted(slopes)]}")
    return hw, min(tls), min(ths)


def _prep_core_inputs(Q, K, sketching_matrix, random_sign):
    """Host-side shard prep: per core one packed [128, INW] array."""
    import ml_dtypes
    X = np.concatenate([np.asarray(Q, np.float32),
                        np.asarray(K, np.float32)], axis=2)  # [B,H,4096,64]
    sk = np.asarray(sketching_matrix).astype(np.int64)       # [B, M, D]
    sign = np.asarray(random_sign, dtype=np.float32)         # [M, D]

    # sign-delta weight matrix W[32m+dl, 32c+dl] = sign[m, 32c+dl]
    W = np.zeros((128, D), dtype=np.float32)
    for m in range(M):
        for c in range(D // 32):
            dl = np.arange(32)
            W[32 * m + dl, 32 * c + dl] = sign[m, 32 * c + dl]

    in_maps = []
    for core in range(NCORES):
        if MM1_FP8:
            # byte-level packing: fp8 X^T/landmarks (pair pr on partitions
            # 32pr..32pr+32, contract feature = kt*32 + p, per-partition
            # layout [kt][n]), then bf16 W; viewed as one bf16 array
            u8 = np.zeros((128, INW_FP8 * 2), dtype=np.uint8)
            for pr in range(PAIRS):
                pair = core * PAIRS + pr
                b, h = divmod(pair, H)
                Xp = X[b, h]                        # [4096, 64]
                lm = Xp[sk[b]]                      # [M, D, 64]
                lmp = lm.reshape(M, D // 32, 32, P).transpose(1, 0, 2, 3)
                lmp = lmp.reshape(L, P)             # [(c, m, dl), 64]
                x8 = Xp.T.astype(ml_dtypes.float8_e4m3fn)   # [64, 4096]
                l8 = lmp.T.astype(ml_dtypes.float8_e4m3fn)  # [64, 1024]
                for kt in range(2):
                    rows = slice(32 * pr, 32 * (pr + 1))
                    u8[rows, kt * SEQ2:(kt + 1) * SEQ2] = \
                        x8[32 * kt:32 * (kt + 1)].view(np.uint8)
                    u8[rows, 2 * SEQ2 + kt * L:2 * SEQ2 + (kt + 1) * L] = \
                        l8[32 * kt:32 * (kt + 1)].view(np.uint8)
            u8[:, 2 * (SEQ2 + L):] = \
                W.astype(ml_dtypes.bfloat16).view(np.uint8)
            in_maps.append({"inp": u8.view(ml_dtypes.bfloat16)})
            continue
        packed = np.zeros((128, INW), dtype=np.float32)
        for pr in range(PAIRS):
            pair = core * PAIRS + pr
            b, h = divmod(pair, H)
            Xp = X[b, h]                            # [4096, 64]
            packed[64 * pr:64 * (pr + 1), 0:SEQ2] = Xp.T
            lm = Xp[sk[b]]                          # [M, D, 64]
            # landmark order l' = 128c + 32m + dl where d = 32c + dl;
            # pair pr's landmark block is zero outside its 64 X-feature
            # rows so the full-contract MM1 drops the other pair's X
            lmp = lm.reshape(M, D // 32, 32, P).transpose(1, 0, 2, 3)
            lmp = lmp.reshape(L, P)                 # [(c, m, dl), 64]
            packed[64 * pr:64 * (pr + 1),
                   SEQ2 + L * pr:SEQ2 + L * (pr + 1)] = lmp.T
        packed[:, SEQ2 + PAIRS * L:INW] = W
        in_maps.append({"inp": packed.astype(ml_dtypes.bfloat16)})
    return in_maps


def kernel(Q, K, sketching_matrix, random_sign):
    in_maps = _prep_core_inputs(Q, K, sketching_matrix, random_sign)
    results = _run_cores(in_maps)
    # unshard: device out [PAIRS, 128, 8192] bf16 (p, h*4096+n) with
    # d = 128h + p  ->  [B, H, 4096, 256] f32
    AS = np.empty((B, H, SEQ2, D), dtype=np.float32)
    for core in range(NCORES):
        o = results[core]["out"]                # [PAIRS, 128, 8192] bf16
        for pr in range(PAIRS):
            pair = core * PAIRS + pr
            b, h = divmod(pair, H)
            op = np.asarray(o[pr]).reshape(128, 2, SEQ2).transpose(1, 0, 2)
            AS[b, h] = op.reshape(D, SEQ2).T.astype(np.float32)
    return AS

